# revision 21
# baseline (speedup 1.0000x reference)
"""DenseGCN (multi-edge-type) Trainium2 kernel.

Data-parallel over batch across 8 NeuronCores (8 graphs per core).

Math (per graph):
  adj_sl = adj with diagonal set to 1 (self loops), per edge type f
  deg[i,f] = clip(sum_j adj_sl[i,j,f], 1)^-0.5
  layer(h) = silu((sum_f D_f adj_sl_f D_f) @ (h W) + b) * mask
  Collapse edge types:  A2[i,j] = sum_f deg[i,f]*adj[i,j,f]*deg[j,f]
  self-loop correction as a diagonal add: A2full = A2 + diag(Cs),
  Cs[i] = sum_f deg[i,f]^2 * (1 - adj[i,i,f]).

Engine plan (per graph, 2-stage software pipeline across graphs):
  DMA:    one casting DMA loads adj fp32->bf16 [P, NCH, N, F] (gpsimd
          SWDGE initiates; HBM bytes unchanged, SBUF halved, no DVE
          rowscale/convert needed).
  GPSIMD: degree row-sums as contiguous pairwise fold tree, levels 1-3
          (512->64 j's), fp32 partials; the idle engine absorbs the
          1M-elem read that used to cost ~160us on ACT+DVE.
  DVE:    fold tail (64->1), deg = rsqrt via fast-inverse-sqrt + 2
          Newton steps (fp32, exact to ~4e-6), hi/lo split of deg,
          one batched TT builds all 32 diag matrices + 4 Cs diag
          blocks, assembly stt (f=1..3), small mask/Cs ops.
  PE:     transpose+deg_i-scale fused: per 128x128 block, 2 matmuls
          (stationary = raw bf16 adj block, moving = diag(hi(deg_i)),
          diag(lo(deg_i))) accumulating in PSUM -- bf16 speed, fp32
          accuracy (hi+lo splits deg into two bf16 values).
          Layers as before: psL[H,i] = sum_cj h^T A2T in f32r.
  ACT:    assembly f=0 scale-copy, Silu activations (single op instead
          of Sigmoid+DVE-mult), x/h0 copies, mask-scaled PSUM->SBUF.
"""

import os

import numpy as np

import concourse.bass as bass
from concourse import bacc, masks, mybir, tile
from concourse.bass_utils import run_bass_kernel_spmd

B, N, F = 64, 512, 4
IN, H, OUT = 64, 128, 16
NCORES = 8
BPC = B // NCORES  # graphs per core
P = 128
NCH = N // P  # 4 chunks of 128 nodes

dt = mybir.dt
AF = mybir.ActivationFunctionType
ALU = mybir.AluOpType
AXL = mybir.AxisListType

MAGIC = 0x5F3759DF

# knobs for A/B on hardware
GP_FOLDS = int(os.environ.get("GCN_GP_FOLDS", "3"))  # fold levels on gpsimd
NEWTON = int(os.environ.get("GCN_NEWTON", "2"))
# AF.Silu is not implemented in CoreSim; set GCN_SILU=0 to validate in sim
USE_SILU = os.environ.get("GCN_SILU", "1") == "1"


def build_nc(n_batches=BPC):
    nc = bacc.Bacc(
        "TRN2", target_bir_lowering=False, debug=False, enable_asserts=False
    )

    adj_d = nc.dram_tensor(
        "adj", [n_batches, N, N, F], dt.float32, kind="ExternalInput"
    )
    x_d = nc.dram_tensor("x", [n_batches, N, IN], dt.float32, kind="ExternalInput")
    mask_d = nc.dram_tensor("mask", [n_batches, N], dt.int32, kind="ExternalInput")
    W0_d = nc.dram_tensor("W0", [IN, H], dt.float32, kind="ExternalInput")
    b0_d = nc.dram_tensor("b0", [H], dt.float32, kind="ExternalInput")
    W1_d = nc.dram_tensor("W1", [H, H], dt.float32, kind="ExternalInput")
    b1_d = nc.dram_tensor("b1", [H], dt.float32, kind="ExternalInput")
    Wl1_d = nc.dram_tensor("Wl1", [H, H], dt.float32, kind="ExternalInput")
    bl1_d = nc.dram_tensor("bl1", [H], dt.float32, kind="ExternalInput")
    Wl2_d = nc.dram_tensor("Wl2", [H, OUT], dt.float32, kind="ExternalInput")
    bl2_d = nc.dram_tensor("bl2", [OUT], dt.float32, kind="ExternalInput")
    out_d = nc.dram_tensor("out", [n_batches, OUT], dt.float32, kind="ExternalOutput")

    f32r = dt.float32r
    bf16 = dt.float16  # fp16: 10 mantissa bits, same PE speed as bf16

    with tile.TileContext(nc) as tc:
        with (
            tc.tile_pool(name="const", bufs=1) as constp,
            tc.tile_pool(name="adjp", bufs=5) as adjp,
            tc.tile_pool(name="a2p", bufs=6) as a2p,
            tc.tile_pool(name="hp", bufs=3) as hp,
            tc.tile_pool(name="smallp", bufs=4) as smallp,
            tc.tile_pool(name="medp", bufs=2) as medp,
            tc.tile_pool(name="dsp", bufs=3) as dsp,
            tc.tile_pool(name="diagp", bufs=2) as diagp,
            tc.tile_pool(name="psA", bufs=2, space="PSUM") as psA,
            tc.tile_pool(name="psB", bufs=3, space="PSUM") as psB,
            tc.tile_pool(name="psC", bufs=1, space="PSUM") as psC,
            tc.tile_pool(name="psD", bufs=1, space="PSUM") as psD,
        ):
            identF = constp.tile([P, P], dt.float32)
            masks.make_identity(nc, identF[:])
            identB = constp.tile([P, P], bf16)
            nc.vector.tensor_copy(identB[:], identF[:])

            W0s = constp.tile([IN, H], dt.float32)
            nc.sync.dma_start(W0s[:], W0_d.ap())
            W1s = constp.tile([H, H], dt.float32)
            nc.sync.dma_start(W1s[:], W1_d.ap())
            Wl1s = constp.tile([H, H], dt.float32)
            nc.sync.dma_start(Wl1s[:], Wl1_d.ap())
            Wl2s = constp.tile([H, OUT], dt.float32)
            nc.sync.dma_start(Wl2s[:], Wl2_d.ap())
            b0col = constp.tile([H, 1], dt.float32)
            nc.sync.dma_start(b0col[:], b0_d.ap().rearrange("(p o) -> p o", o=1))
            b1col = constp.tile([H, 1], dt.float32)
            nc.sync.dma_start(b1col[:], b1_d.ap().rearrange("(p o) -> p o", o=1))
            bl1c = constp.tile([H, 1], dt.float32)
            nc.sync.dma_start(bl1c[:], bl1_d.ap().rearrange("(p o) -> p o", o=1))
            bl2c = constp.tile([OUT, 1], dt.float32)
            nc.sync.dma_start(bl2c[:], bl2_d.ap().rearrange("(p o) -> p o", o=1))
            gAll = constp.tile([1, n_batches * H], dt.float32)
            outS = constp.tile([OUT, n_batches], dt.float32)

            def dma_issue(b):
                st = {}
                # whole-graph adj as bf16 (DMA converts): [i-part, ci, j, f]
                adjB = adjp.tile([P, NCH, N, F], bf16, tag="adjB")
                nc.gpsimd.dma_start(
                    adjB[:],
                    adj_d.ap()[b].rearrange("(c p) j f -> p c j f", p=P),
                )
                st["adjB"] = adjB
                diagN = smallp.tile([P, NCH, F], dt.float32, tag="diag")
                nc.sync.dma_start(
                    diagN[:],
                    bass.AP(
                        tensor=adj_d,
                        offset=b * N * N * F,
                        ap=[[(N * F + F), P], [(N * F + F) * P, NCH], [1, F]],
                    ),
                )
                xb = smallp.tile([P, NCH, IN], dt.float32, tag="xb")
                nc.sync.dma_start(
                    xb[:], x_d.ap()[b].rearrange("(c p) d -> p c d", p=P)
                )
                mi = smallp.tile([P, NCH], dt.int32, tag="mi")
                nc.sync.dma_start(
                    mi[:], mask_d.ap()[b].rearrange("(c p) -> p c", p=P)
                )
                st["diagN"] = diagN
                st["xb"] = xb
                st["mi"] = mi
                return st

            def stage_folds(b, st):
                # gpsimd-only: fold levels 1-3 (512 -> 64 j's).  Runs one
                # pipeline stage ahead of stage_degmath so the DVE tail
                # never waits on gpsimd mid-queue.  Early-level partials in
                # fp16 (error contribution ~1e-4 on a ~256 sum).
                adjB = st["adjB"]
                hdt = dt.float16
                ds1 = dsp.tile([P, NCH, (N // 4) * F], hdt, tag="ds1")
                half = N // 2
                for ci in range(NCH):
                    t1 = dsp.tile([P, half * F], hdt, tag="t1")
                    nc.gpsimd.tensor_tensor(
                        t1[:],
                        adjB[:, ci, 0:half, :].rearrange("p j f -> p (j f)"),
                        adjB[:, ci, half:N, :].rearrange("p j f -> p (j f)"),
                        ALU.add,
                    )
                    nc.gpsimd.tensor_tensor(
                        ds1[:, ci, :],
                        t1[:, 0 : (half // 2) * F],
                        t1[:, (half // 2) * F : half * F],
                        ALU.add,
                    )
                w = N // 4
                ds3 = dsp.tile([P, NCH, (w // 2) * F], hdt, tag="ds3")
                nc.gpsimd.tensor_tensor(
                    ds3[:],
                    ds1[:, :, 0 : (w // 2) * F],
                    ds1[:, :, (w // 2) * F : w * F],
                    ALU.add,
                )
                st["ds3"] = ds3

            def stage_degmath(b, st):
                adjB = st["adjB"]
                diagN = st["diagN"]
                xb = st["xb"]
                mi = st["mi"]
                maskb = smallp.tile([P, NCH], dt.float32, tag="maskb")
                nc.vector.tensor_copy(maskb[:], mi[:])
                st["maskb"] = maskb
                maskdiv = smallp.tile([P, NCH], dt.float32, tag="md")
                nc.vector.tensor_scalar_mul(maskdiv[:], maskb[:], 1.0 / N)
                st["maskdiv"] = maskdiv

                # DVE fold tail 64 -> 1 j's (fp16 2x for the wide levels,
                # fp32 from width 32 down)
                cur = st.pop("ds3")
                w = N // 8
                while w > 1:
                    odt = dt.float16 if w > 16 else dt.float32
                    nxt = dsp.tile([P, NCH, (w // 2) * F], odt, tag=f"ds{w}")
                    nc.vector.tensor_tensor(
                        nxt[:],
                        cur[:, :, 0 : (w // 2) * F],
                        cur[:, :, (w // 2) * F : w * F],
                        ALU.add,
                    )
                    cur, w = nxt, w // 2
                degsum = cur  # [P, NCH, F]

                # dtmp = max(degsum + 1 - diag, 1)
                dtmp = smallp.tile([P, NCH, F], dt.float32, tag="dtmp")
                nc.vector.tensor_tensor(
                    dtmp[:], degsum[:], diagN[:], ALU.subtract
                )
                nc.vector.tensor_scalar(dtmp[:], dtmp[:], 1.0, 1.0, ALU.add, ALU.max)
                # deg = dtmp^-0.5 fast-inverse-sqrt + Newton (DVE, fp32)
                ti = smallp.tile([P, NCH, F], dt.int32, tag="ti")
                nc.vector.tensor_scalar(
                    ti[:], dtmp[:].bitcast(dt.int32), 1, None, ALU.arith_shift_right
                )
                nc.vector.tensor_scalar(ti[:], ti[:], -1, MAGIC, ALU.mult, ALU.add)
                e = smallp.tile([P, NCH, F], dt.float32, tag="nwt")
                deg = ti[:].bitcast(dt.float32)
                for _ in range(NEWTON):
                    nc.vector.tensor_tensor(e[:], dtmp[:], deg, ALU.mult)
                    nc.vector.tensor_tensor(e[:], e[:], deg, ALU.mult)
                    nc.vector.tensor_scalar(
                        e[:], e[:], -0.5, 1.5, ALU.mult, ALU.add
                    )
                    nc.vector.tensor_tensor(deg, deg, e[:], ALU.mult)
                st["deg"] = deg

                # Cs = sum_f deg^2 * (1 - diag)
                om = smallp.tile([P, NCH, F], dt.float32, tag="om")
                nc.scalar.activation(
                    om[:], diagN[:], AF.Copy, scale=-1.0, bias=1.0
                )
                csf = smallp.tile([P, NCH, F], dt.float32, tag="csf")
                nc.vector.tensor_tensor(csf[:], deg, deg, ALU.mult)
                nc.vector.tensor_tensor(csf[:], csf[:], om[:], ALU.mult)
                Cs = smallp.tile([P, NCH], dt.float32, tag="Cs")
                nc.vector.tensor_reduce(Cs[:], csf[:], axis=AXL.X, op=ALU.add)

                # hi/lo split of deg into bf16 pair (exact to ~2^-16)
                NG = NCH * F
                degB = smallp.tile([P, NCH, F], bf16, tag="degB")
                nc.scalar.copy(degB[:], deg)
                hiF = smallp.tile([P, NCH, F], dt.float32, tag="hiF")
                nc.scalar.copy(hiF[:], degB[:])
                loF = smallp.tile([P, NCH, F], dt.float32, tag="loF")
                nc.vector.tensor_tensor(loF[:], deg, hiF[:], ALU.subtract)

                # diagonal build: diagHL[p, hl, g, q] = ident[p, q] *
                # (hl ? lo : hi)[p, g], g = (ci, f).  hi half: 16 ACT
                # scale-copies (ACT has slack); lo half: one batched DVE TT.
                diagHL = diagp.tile([P, 2, NG, P], bf16, tag="diagHL")
                for ci in range(NCH):
                    for f in range(F):
                        nc.scalar.activation(
                            diagHL[:, 0, ci * F + f, :],
                            identB[:],
                            AF.Copy,
                            scale=hiF[:, ci, f : f + 1],
                        )
                loFf = loF[:].rearrange("p c f -> p (c f)")
                nc.vector.tensor_tensor(
                    diagHL[:, 1, :, :],
                    identB[:, None, :].to_broadcast([P, NG, P]),
                    loFf[:, :, None].to_broadcast([P, NG, P]),
                    ALU.mult,
                )
                st["diagHL"] = diagHL
                # Cs diagonal blocks (bf16)
                csdB = diagp.tile([P, NCH, P], bf16, tag="csdB")
                nc.vector.tensor_tensor(
                    csdB[:],
                    identB[:, None, :].to_broadcast([P, NCH, P]),
                    Cs[:, :, None].to_broadcast([P, NCH, P]),
                    ALU.mult,
                )
                st["csdB"] = csdB

                # h0 = x @ W0 (natural [j, H] layout, exact fp32)
                psX = psC.tile([IN, N], dt.float32, tag="px")
                for ci in range(NCH):
                    nc.tensor.transpose(
                        psX[:, ci * P : (ci + 1) * P], xb[:, ci, :], identF[:]
                    )
                xTs = medp.tile([IN, N], dt.float32, tag="xTs")
                nc.scalar.copy(xTs[:], psX[:])
                psH0 = psC.tile([P, NCH, H], dt.float32, tag="px")
                for ci in range(NCH):
                    nc.tensor.matmul(
                        psH0[:, ci, :],
                        xTs[:, ci * P : (ci + 1) * P],
                        W0s[:],
                        start=True,
                        stop=True,
                    )
                h0 = hp.tile([P, NCH, H], f32r, tag="h0")
                nc.scalar.copy(h0[:], psH0[:])
                st["h0"] = h0

            def stage_compute(b, st):
                adjB = st["adjB"]
                deg = st["deg"]
                diagHL = st["diagHL"]
                csdB = st["csdB"]
                maskb = st["maskb"]
                maskdiv = st["maskdiv"]

                # transpose + deg_i scale (hi/lo diag matmuls) + assemble
                # A2T [j part, i free] (+ Cs diag)
                # Per cj, two partial accumulators: accA = deg0*BT0 +
                # deg1*BT1 (+ Cs diag), accB = deg2*BT2 + deg3*BT3.  The
                # f=0/f=2 scale-copies run on ACT, f=1/f=3 stt on DVE; the
                # layer matmuls accumulate both halves in PSUM, so no final
                # merge op is needed.
                A2Ta, A2Tb = [], []
                for cj in range(NCH):
                    accA = a2p.tile([P, N], f32r, tag="A2Ta")
                    accB = a2p.tile([P, N], f32r, tag="A2Tb")
                    for f in range(F):
                        acc = accA if f < 2 else accB
                        BT = psA.tile([P, N], dt.float32, tag="BT")
                        for ci in range(NCH):
                            blk = adjB[:, ci, cj * P : (cj + 1) * P, f]
                            out = BT[:, ci * P : (ci + 1) * P]
                            g = ci * F + f
                            nc.tensor.matmul(
                                out, blk, diagHL[:, 0, g, :],
                                start=True, stop=False,
                            )
                            nc.tensor.matmul(
                                out, blk, diagHL[:, 1, g, :],
                                start=False, stop=True,
                            )
                        if f % 2 == 0:
                            nc.scalar.activation(
                                acc[:], BT[:], AF.Copy, scale=deg[:, cj, f : f + 1]
                            )
                        else:
                            nc.vector.scalar_tensor_tensor(
                                acc[:], BT[:], deg[:, cj, f : f + 1], acc[:],
                                op0=ALU.mult, op1=ALU.add,
                            )
                    nc.vector.tensor_tensor(
                        accA[:, cj * P : (cj + 1) * P],
                        accA[:, cj * P : (cj + 1) * P],
                        csdB[:, cj, :],
                        ALU.add,
                    )
                    A2Ta.append(accA)
                    A2Tb.append(accB)

                # two GCN layers, transposed [H, i] layout
                hw = st["h0"]
                for l in range(2):
                    psL = psB.tile([H, N], dt.float32, tag="mm")
                    for cj in range(NCH):
                        for A2T in (A2Ta, A2Tb):
                            nc.tensor.matmul(
                                psL[:],
                                hw[:, cj, :],
                                A2T[cj][:],
                                start=(cj == 0 and A2T is A2Ta),
                                stop=(cj == NCH - 1 and A2T is A2Tb),
                            )
                    if l == 0:
                        h1T = medp.tile([H, N], dt.float32, tag="h1T")
                        if USE_SILU:
                            nc.scalar.activation(
                                h1T[:], psL[:], AF.Silu, bias=b0col[:, 0:1]
                            )
                        else:
                            sg1 = medp.tile([H, N], dt.float32, tag="sg1")
                            nc.scalar.activation(
                                sg1[:], psL[:], AF.Sigmoid, bias=b0col[:, 0:1]
                            )
                            nc.vector.scalar_tensor_tensor(
                                h1T[:], psL[:], b0col[:, 0:1], sg1[:],
                                op0=ALU.add, op1=ALU.mult,
                            )
                        psW = psB.tile([P, NCH, H], dt.float32, tag="mm")
                        for cj in range(NCH):
                            nc.tensor.matmul(
                                psW[:, cj, :],
                                h1T[:, cj * P : (cj + 1) * P],
                                W1s[:],
                                start=True,
                                stop=True,
                            )
                        hw = hp.tile([P, NCH, H], f32r, tag="hw")
                        for cj in range(NCH):
                            nc.scalar.activation(
                                hw[:, cj, :],
                                psW[:, cj, :],
                                AF.Copy,
                                scale=maskb[:, cj : cj + 1],
                            )
                    else:
                        L2s = medp.tile([H, N], dt.float32, tag="L2s")
                        nc.scalar.activation(
                            L2s[:], psL[:], AF.Identity, bias=b1col[:, 0:1]
                        )
                        psN = psB.tile([P, NCH, H], dt.float32, tag="mm")
                        for ci in range(NCH):
                            nc.tensor.transpose(
                                psN[:, ci, :],
                                L2s[:, ci * P : (ci + 1) * P],
                                identF[:],
                            )
                        h2 = hp.tile([P, NCH, H], dt.float32, tag="h2")
                        if USE_SILU:
                            nc.scalar.activation(h2[:], psN[:], AF.Silu)
                        else:
                            sg2 = hp.tile([P, NCH, H], dt.float32, tag="sg2")
                            nc.scalar.activation(sg2[:], psN[:], AF.Sigmoid)
                            nc.vector.tensor_tensor(
                                h2[:], psN[:], sg2[:], ALU.mult
                            )

                # masked mean pool; per-graph embedding -> gAll row
                psG = psD.tile([1, H], dt.float32, tag="head")
                for ci in range(NCH):
                    nc.tensor.matmul(
                        psG[:],
                        maskdiv[:, ci : ci + 1],
                        h2[:, ci, :],
                        start=(ci == 0),
                        stop=(ci == NCH - 1),
                    )
                nc.scalar.copy(gAll[:, b * H : (b + 1) * H], psG[:])

            # 3-stage software pipeline: dma(b) | deg-chain(b-1) | compute(b-2)
            # 5-stage pipeline: dma(b) | transfer in flight | folds(b-1)
            # on gpsimd | deg-math(b-2) on DVE/ACT | compute(b-3).  Each
            # stage's inputs were produced a full iteration earlier, so no
            # engine ever stalls mid-queue on another engine's current
            # work; emission order puts compute first in every queue.
            sts = {}
            for b in range(n_batches + 3):
                if b >= 3:
                    stage_compute(b - 3, sts.pop(b - 3))
                if 1 <= b < n_batches + 1:
                    stage_folds(b - 1, sts[b - 1])
                if b < n_batches:
                    sts[b] = dma_issue(b)
                if 2 <= b < n_batches + 2:
                    stage_degmath(b - 2, sts[b - 2])

            # batched MLP head over all graphs: [H, B] columns
            psGT = psD.tile([H, n_batches], dt.float32, tag="head2")
            for b in range(n_batches):
                nc.tensor.transpose(
                    psGT[:, b : b + 1],
                    gAll[:, b * H : (b + 1) * H],
                    identF[0:1, 0:1],
                )
            gT = constp.tile([H, n_batches], dt.float32)
            nc.scalar.copy(gT[:], psGT[:])
            psH1 = psD.tile([H, n_batches], dt.float32, tag="head2")
            nc.tensor.matmul(psH1[:], Wl1s[:], gT[:], start=True, stop=True)
            g1 = constp.tile([H, n_batches], dt.float32)
            if USE_SILU:
                nc.scalar.activation(g1[:], psH1[:], AF.Silu, bias=bl1c[:, 0:1])
            else:
                g1s = constp.tile([H, n_batches], dt.float32)
                nc.scalar.activation(
                    g1s[:], psH1[:], AF.Sigmoid, bias=bl1c[:, 0:1]
                )
                nc.vector.scalar_tensor_tensor(
                    g1[:], psH1[:], bl1c[:, 0:1], g1s[:],
                    op0=ALU.add, op1=ALU.mult,
                )
            psO = psD.tile([OUT, n_batches], dt.float32, tag="head2")
            nc.tensor.matmul(psO[:], Wl2s[:], g1[:], start=True, stop=True)
            nc.scalar.activation(outS[:], psO[:], AF.Identity, bias=bl2c[:, 0:1])

            nc.sync.dma_start(out_d.ap().rearrange("b c -> c b"), outS[:])

    nc.compile()
    return nc


_NC_CACHE = {}


def _get_nc(n_batches=BPC):
    if n_batches not in _NC_CACHE:
        _NC_CACHE[n_batches] = build_nc(n_batches)
    return _NC_CACHE[n_batches]


def make_in_maps(x, adj, mask, W0, b0, W1, b1, Wl1, bl1, Wl2, bl2):
    ws = dict(
        W0=np.ascontiguousarray(W0, np.float32),
        b0=np.ascontiguousarray(b0, np.float32),
        W1=np.ascontiguousarray(W1, np.float32),
        b1=np.ascontiguousarray(b1, np.float32),
        Wl1=np.ascontiguousarray(Wl1, np.float32),
        bl1=np.ascontiguousarray(bl1, np.float32),
        Wl2=np.ascontiguousarray(Wl2, np.float32),
        bl2=np.ascontiguousarray(bl2, np.float32),
    )
    in_maps = []
    for c in range(NCORES):
        sl = slice(c * BPC, (c + 1) * BPC)
        m = dict(
            adj=np.ascontiguousarray(adj[sl], np.float32),
            x=np.ascontiguousarray(x[sl], np.float32),
            mask=np.ascontiguousarray(mask[sl], np.int32),
        )
        m.update(ws)
        in_maps.append(m)
    return in_maps


def kernel(x, adj, mask, W0, b0, W1, b1, Wl1, bl1, Wl2, bl2, **kw):
    nc = _get_nc()
    in_maps = make_in_maps(x, adj, mask, W0, b0, W1, b1, Wl1, bl1, Wl2, bl2)
    res = run_bass_kernel_spmd(nc, in_maps, core_ids=list(range(NCORES)))
    out = np.concatenate([res.results[c]["out"] for c in range(NCORES)], axis=0)
    return out.astype(np.float32)


# revision 22
# speedup vs baseline: 1.0798x; 1.0798x over previous
"""DenseGCN (multi-edge-type) Trainium2 kernel.

Data-parallel over batch across 8 NeuronCores (8 graphs per core).

Math (per graph):
  adj_sl = adj with diagonal set to 1 (self loops), per edge type f
  deg[i,f] = clip(sum_j adj_sl[i,j,f], 1)^-0.5
  layer(h) = silu((sum_f D_f adj_sl_f D_f) @ (h W) + b) * mask
  Collapse edge types:  A2[i,j] = sum_f deg[i,f]*adj[i,j,f]*deg[j,f]
  self-loop correction as a diagonal add: A2full = A2 + diag(Cs),
  Cs[i] = sum_f deg[i,f]^2 * (1 - adj[i,i,f]).

Engine plan (per graph, 2-stage software pipeline across graphs):
  DMA:    one casting DMA loads adj fp32->bf16 [P, NCH, N, F] (gpsimd
          SWDGE initiates; HBM bytes unchanged, SBUF halved, no DVE
          rowscale/convert needed).
  GPSIMD: degree row-sums as contiguous pairwise fold tree, levels 1-3
          (512->64 j's), fp32 partials; the idle engine absorbs the
          1M-elem read that used to cost ~160us on ACT+DVE.
  DVE:    fold tail (64->1), deg = rsqrt via fast-inverse-sqrt + 2
          Newton steps (fp32, exact to ~4e-6), hi/lo split of deg,
          one batched TT builds all 32 diag matrices + 4 Cs diag
          blocks, assembly stt (f=1..3), small mask/Cs ops.
  PE:     transpose+deg_i-scale fused: per 128x128 block, 2 matmuls
          (stationary = raw bf16 adj block, moving = diag(hi(deg_i)),
          diag(lo(deg_i))) accumulating in PSUM -- bf16 speed, fp32
          accuracy (hi+lo splits deg into two bf16 values).
          Layers as before: psL[H,i] = sum_cj h^T A2T in f32r.
  ACT:    assembly f=0 scale-copy, Silu activations (single op instead
          of Sigmoid+DVE-mult), x/h0 copies, mask-scaled PSUM->SBUF.
"""

import os

import numpy as np

import concourse.bass as bass
from concourse import bacc, masks, mybir, tile
from concourse.bass_utils import run_bass_kernel_spmd

B, N, F = 64, 512, 4
IN, H, OUT = 64, 128, 16
NCORES = 8
BPC = B // NCORES  # graphs per core
P = 128
NCH = N // P  # 4 chunks of 128 nodes

dt = mybir.dt
AF = mybir.ActivationFunctionType
ALU = mybir.AluOpType
AXL = mybir.AxisListType

MAGIC = 0x5F3759DF

# knobs for A/B on hardware
GP_FOLDS = int(os.environ.get("GCN_GP_FOLDS", "3"))  # fold levels on gpsimd
NEWTON = int(os.environ.get("GCN_NEWTON", "2"))
# AF.Silu is not implemented in CoreSim; set GCN_SILU=0 to validate in sim
USE_SILU = os.environ.get("GCN_SILU", "1") == "1"


def build_nc(n_batches=BPC):
    nc = bacc.Bacc(
        "TRN2", target_bir_lowering=False, debug=False, enable_asserts=False
    )

    adj_d = nc.dram_tensor(
        "adj", [n_batches, N, N, F], dt.float32, kind="ExternalInput"
    )
    x_d = nc.dram_tensor("x", [n_batches, N, IN], dt.float32, kind="ExternalInput")
    mask_d = nc.dram_tensor("mask", [n_batches, N], dt.int32, kind="ExternalInput")
    W0_d = nc.dram_tensor("W0", [IN, H], dt.float32, kind="ExternalInput")
    b0_d = nc.dram_tensor("b0", [H], dt.float32, kind="ExternalInput")
    W1_d = nc.dram_tensor("W1", [H, H], dt.float32, kind="ExternalInput")
    b1_d = nc.dram_tensor("b1", [H], dt.float32, kind="ExternalInput")
    Wl1_d = nc.dram_tensor("Wl1", [H, H], dt.float32, kind="ExternalInput")
    bl1_d = nc.dram_tensor("bl1", [H], dt.float32, kind="ExternalInput")
    Wl2_d = nc.dram_tensor("Wl2", [H, OUT], dt.float32, kind="ExternalInput")
    bl2_d = nc.dram_tensor("bl2", [OUT], dt.float32, kind="ExternalInput")
    out_d = nc.dram_tensor("out", [n_batches, OUT], dt.float32, kind="ExternalOutput")

    f32r = dt.float32r
    bf16 = dt.float16  # fp16: 10 mantissa bits, same PE speed as bf16

    with tile.TileContext(nc) as tc:
        with (
            tc.tile_pool(name="const", bufs=1) as constp,
            tc.tile_pool(name="adjp", bufs=5) as adjp,
            tc.tile_pool(name="a2p", bufs=2 * NCH) as a2p,
            tc.tile_pool(name="hp", bufs=3) as hp,
            tc.tile_pool(name="smallp", bufs=4) as smallp,
            tc.tile_pool(name="medp", bufs=2) as medp,
            tc.tile_pool(name="dsp", bufs=3) as dsp,
            tc.tile_pool(name="dsl", bufs=2) as dsl,
            tc.tile_pool(name="diagp", bufs=2) as diagp,
            tc.tile_pool(name="psA", bufs=2, space="PSUM") as psA,
            tc.tile_pool(name="psB", bufs=3, space="PSUM") as psB,
            tc.tile_pool(name="psC", bufs=1, space="PSUM") as psC,
            tc.tile_pool(name="psD", bufs=1, space="PSUM") as psD,
        ):
            identF = constp.tile([P, P], dt.float32)
            masks.make_identity(nc, identF[:])
            identB = constp.tile([P, P], bf16)
            nc.vector.tensor_copy(identB[:], identF[:])

            W0s = constp.tile([IN, H], dt.float32)
            nc.sync.dma_start(W0s[:], W0_d.ap())
            W1s = constp.tile([H, H], dt.float32)
            nc.sync.dma_start(W1s[:], W1_d.ap())
            Wl1s = constp.tile([H, H], dt.float32)
            nc.sync.dma_start(Wl1s[:], Wl1_d.ap())
            Wl2s = constp.tile([H, OUT], dt.float32)
            nc.sync.dma_start(Wl2s[:], Wl2_d.ap())
            b0col = constp.tile([H, 1], dt.float32)
            nc.sync.dma_start(b0col[:], b0_d.ap().rearrange("(p o) -> p o", o=1))
            b1col = constp.tile([H, 1], dt.float32)
            nc.sync.dma_start(b1col[:], b1_d.ap().rearrange("(p o) -> p o", o=1))
            bl1c = constp.tile([H, 1], dt.float32)
            nc.sync.dma_start(bl1c[:], bl1_d.ap().rearrange("(p o) -> p o", o=1))
            bl2c = constp.tile([OUT, 1], dt.float32)
            nc.sync.dma_start(bl2c[:], bl2_d.ap().rearrange("(p o) -> p o", o=1))
            gAll = constp.tile([1, n_batches * H], dt.float32)
            outS = constp.tile([OUT, n_batches], dt.float32)

            def dma_issue(b):
                st = {}
                # whole-graph adj as bf16 (DMA converts): [i-part, ci, j, f]
                adjB = adjp.tile([P, NCH, N, F], bf16, tag="adjB")
                nc.gpsimd.dma_start(
                    adjB[:],
                    adj_d.ap()[b].rearrange("(c p) j f -> p c j f", p=P),
                )
                st["adjB"] = adjB
                diagN = smallp.tile([P, NCH, F], dt.float32, tag="diag")
                nc.sync.dma_start(
                    diagN[:],
                    bass.AP(
                        tensor=adj_d,
                        offset=b * N * N * F,
                        ap=[[(N * F + F), P], [(N * F + F) * P, NCH], [1, F]],
                    ),
                )
                xb = smallp.tile([P, NCH, IN], dt.float32, tag="xb")
                nc.sync.dma_start(
                    xb[:], x_d.ap()[b].rearrange("(c p) d -> p c d", p=P)
                )
                mi = smallp.tile([P, NCH], dt.int32, tag="mi")
                nc.sync.dma_start(
                    mi[:], mask_d.ap()[b].rearrange("(c p) -> p c", p=P)
                )
                st["diagN"] = diagN
                st["xb"] = xb
                st["mi"] = mi
                return st

            def stage_folds(b, st):
                # gpsimd-only: fold levels 1-3 (512 -> 64 j's).  Runs one
                # pipeline stage ahead of stage_degmath so the DVE tail
                # never waits on gpsimd mid-queue.  Early-level partials in
                # fp16 (error contribution ~1e-4 on a ~256 sum).
                adjB = st["adjB"]
                hdt = dt.float16
                ds1 = dsl.tile([P, NCH, (N // 4) * F], hdt, tag="ds1")
                half = N // 2
                for ci in range(NCH):
                    t1 = dsl.tile([P, half * F], hdt, tag="t1")
                    nc.gpsimd.tensor_tensor(
                        t1[:],
                        adjB[:, ci, 0:half, :].rearrange("p j f -> p (j f)"),
                        adjB[:, ci, half:N, :].rearrange("p j f -> p (j f)"),
                        ALU.add,
                    )
                    nc.gpsimd.tensor_tensor(
                        ds1[:, ci, :],
                        t1[:, 0 : (half // 2) * F],
                        t1[:, (half // 2) * F : half * F],
                        ALU.add,
                    )
                w = N // 4
                ds3 = dsp.tile([P, NCH, (w // 2) * F], hdt, tag="ds3")
                nc.gpsimd.tensor_tensor(
                    ds3[:],
                    ds1[:, :, 0 : (w // 2) * F],
                    ds1[:, :, (w // 2) * F : w * F],
                    ALU.add,
                )
                st["ds3"] = ds3

            def stage_degmath(b, st):
                adjB = st["adjB"]
                diagN = st["diagN"]
                xb = st["xb"]
                mi = st["mi"]
                maskb = smallp.tile([P, NCH], dt.float32, tag="maskb")
                nc.vector.tensor_copy(maskb[:], mi[:])
                st["maskb"] = maskb
                maskdiv = smallp.tile([P, NCH], dt.float32, tag="md")
                nc.vector.tensor_scalar_mul(maskdiv[:], maskb[:], 1.0 / N)
                st["maskdiv"] = maskdiv

                # DVE fold tail 64 -> 1 j's (fp16 2x for the wide levels,
                # fp32 from width 32 down)
                cur = st.pop("ds3")
                w = N // 8
                while w > 1:
                    odt = dt.float16 if w > 16 else dt.float32
                    nxt = dsl.tile([P, NCH, (w // 2) * F], odt, tag=f"ds{w}")
                    nc.vector.tensor_tensor(
                        nxt[:],
                        cur[:, :, 0 : (w // 2) * F],
                        cur[:, :, (w // 2) * F : w * F],
                        ALU.add,
                    )
                    cur, w = nxt, w // 2
                degsum = cur  # [P, NCH, F]

                # dtmp = max(degsum + 1 - diag, 1)
                dtmp = smallp.tile([P, NCH, F], dt.float32, tag="dtmp")
                nc.vector.tensor_tensor(
                    dtmp[:], degsum[:], diagN[:], ALU.subtract
                )
                nc.vector.tensor_scalar(dtmp[:], dtmp[:], 1.0, 1.0, ALU.add, ALU.max)
                # deg = dtmp^-0.5 fast-inverse-sqrt + Newton (DVE, fp32)
                ti = smallp.tile([P, NCH, F], dt.int32, tag="ti")
                nc.vector.tensor_scalar(
                    ti[:], dtmp[:].bitcast(dt.int32), 1, None, ALU.arith_shift_right
                )
                nc.vector.tensor_scalar(ti[:], ti[:], -1, MAGIC, ALU.mult, ALU.add)
                e = smallp.tile([P, NCH, F], dt.float32, tag="nwt")
                deg = ti[:].bitcast(dt.float32)
                for _ in range(NEWTON):
                    nc.vector.tensor_tensor(e[:], dtmp[:], deg, ALU.mult)
                    nc.vector.tensor_tensor(e[:], e[:], deg, ALU.mult)
                    nc.vector.tensor_scalar(
                        e[:], e[:], -0.5, 1.5, ALU.mult, ALU.add
                    )
                    nc.vector.tensor_tensor(deg, deg, e[:], ALU.mult)
                st["deg"] = deg

                # Cs = sum_f deg^2 * (1 - diag)
                om = smallp.tile([P, NCH, F], dt.float32, tag="om")
                nc.scalar.activation(
                    om[:], diagN[:], AF.Copy, scale=-1.0, bias=1.0
                )
                csf = smallp.tile([P, NCH, F], dt.float32, tag="csf")
                nc.vector.tensor_tensor(csf[:], deg, deg, ALU.mult)
                nc.vector.tensor_tensor(csf[:], csf[:], om[:], ALU.mult)
                Cs = smallp.tile([P, NCH], dt.float32, tag="Cs")
                nc.vector.tensor_reduce(Cs[:], csf[:], axis=AXL.X, op=ALU.add)

                # hi/lo split of deg into bf16 pair (exact to ~2^-16)
                NG = NCH * F
                degB = smallp.tile([P, NCH, F], bf16, tag="degB")
                nc.scalar.copy(degB[:], deg)
                hiF = smallp.tile([P, NCH, F], dt.float32, tag="hiF")
                nc.scalar.copy(hiF[:], degB[:])
                loF = smallp.tile([P, NCH, F], dt.float32, tag="loF")
                nc.vector.tensor_tensor(loF[:], deg, hiF[:], ALU.subtract)

                # diagonal build: diagHL[p, hl, g, q] = ident[p, q] *
                # (hl ? lo : hi)[p, g], g = (ci, f).  hi half: 16 ACT
                # scale-copies (ACT has slack); lo half: one batched DVE TT.
                diagHL = diagp.tile([P, 2, NG, P], bf16, tag="diagHL")
                for ci in range(NCH):
                    for f in range(F):
                        nc.scalar.activation(
                            diagHL[:, 0, ci * F + f, :],
                            identB[:],
                            AF.Copy,
                            scale=hiF[:, ci, f : f + 1],
                        )
                loFf = loF[:].rearrange("p c f -> p (c f)")
                nc.vector.tensor_tensor(
                    diagHL[:, 1, :, :],
                    identB[:, None, :].to_broadcast([P, NG, P]),
                    loFf[:, :, None].to_broadcast([P, NG, P]),
                    ALU.mult,
                )
                st["diagHL"] = diagHL
                # Cs diagonal blocks (bf16)
                csdB = diagp.tile([P, NCH, P], bf16, tag="csdB")
                nc.vector.tensor_tensor(
                    csdB[:],
                    identB[:, None, :].to_broadcast([P, NCH, P]),
                    Cs[:, :, None].to_broadcast([P, NCH, P]),
                    ALU.mult,
                )
                st["csdB"] = csdB

                # h0 = x @ W0 (natural [j, H] layout, exact fp32)
                psX = psC.tile([IN, N], dt.float32, tag="px")
                for ci in range(NCH):
                    nc.tensor.transpose(
                        psX[:, ci * P : (ci + 1) * P], xb[:, ci, :], identF[:]
                    )
                xTs = medp.tile([IN, N], dt.float32, tag="xTs")
                nc.scalar.copy(xTs[:], psX[:])
                psH0 = psC.tile([P, NCH, H], dt.float32, tag="px")
                for ci in range(NCH):
                    nc.tensor.matmul(
                        psH0[:, ci, :],
                        xTs[:, ci * P : (ci + 1) * P],
                        W0s[:],
                        start=True,
                        stop=True,
                    )
                h0 = hp.tile([P, NCH, H], f32r, tag="h0")
                nc.scalar.copy(h0[:], psH0[:])
                st["h0"] = h0

            def stage_compute(b, st):
                adjB = st["adjB"]
                deg = st["deg"]
                diagHL = st["diagHL"]
                csdB = st["csdB"]
                maskb = st["maskb"]
                maskdiv = st["maskdiv"]

                # transpose + deg_i scale (hi/lo diag matmuls) + assemble
                # A2T [j part, i free] (+ Cs diag)
                # Per cj, two partial accumulators: accA = deg0*BT0 +
                # deg1*BT1 (+ Cs diag), accB = deg2*BT2 + deg3*BT3.  The
                # f=0/f=2 scale-copies run on ACT, f=1/f=3 stt on DVE; the
                # layer matmuls accumulate both halves in PSUM, so no final
                # merge op is needed.
                A2Ta, A2Tb = [], []
                for cj in range(NCH):
                    accA = a2p.tile([P, N], f32r, tag="A2Ta")
                    accB = a2p.tile([P, N], f32r, tag="A2Tb")
                    for f in range(F):
                        acc = accA if f < 2 else accB
                        BT = psA.tile([P, N], dt.float32, tag="BT")
                        for ci in range(NCH):
                            blk = adjB[:, ci, cj * P : (cj + 1) * P, f]
                            out = BT[:, ci * P : (ci + 1) * P]
                            g = ci * F + f
                            nc.tensor.matmul(
                                out, blk, diagHL[:, 0, g, :],
                                start=True, stop=False,
                            )
                            nc.tensor.matmul(
                                out, blk, diagHL[:, 1, g, :],
                                start=False, stop=True,
                            )
                        if f % 2 == 0:
                            nc.scalar.activation(
                                acc[:], BT[:], AF.Copy, scale=deg[:, cj, f : f + 1]
                            )
                        else:
                            nc.vector.scalar_tensor_tensor(
                                acc[:], BT[:], deg[:, cj, f : f + 1], acc[:],
                                op0=ALU.mult, op1=ALU.add,
                            )
                    nc.vector.tensor_tensor(
                        accA[:, cj * P : (cj + 1) * P],
                        accA[:, cj * P : (cj + 1) * P],
                        csdB[:, cj, :],
                        ALU.add,
                    )
                    A2Ta.append(accA)
                    A2Tb.append(accB)

                # two GCN layers, transposed [H, i] layout
                hw = st["h0"]
                for l in range(2):
                    psL = psB.tile([H, N], dt.float32, tag="mm")
                    for cj in range(NCH):
                        for A2T in (A2Ta, A2Tb):
                            nc.tensor.matmul(
                                psL[:],
                                hw[:, cj, :],
                                A2T[cj][:],
                                start=(cj == 0 and A2T is A2Ta),
                                stop=(cj == NCH - 1 and A2T is A2Tb),
                            )
                    if l == 0:
                        h1T = medp.tile([H, N], dt.float32, tag="h1T")
                        if USE_SILU:
                            nc.scalar.activation(
                                h1T[:], psL[:], AF.Silu, bias=b0col[:, 0:1]
                            )
                        else:
                            sg1 = medp.tile([H, N], dt.float32, tag="sg1")
                            nc.scalar.activation(
                                sg1[:], psL[:], AF.Sigmoid, bias=b0col[:, 0:1]
                            )
                            nc.vector.scalar_tensor_tensor(
                                h1T[:], psL[:], b0col[:, 0:1], sg1[:],
                                op0=ALU.add, op1=ALU.mult,
                            )
                        psW = psB.tile([P, NCH, H], dt.float32, tag="mm")
                        for cj in range(NCH):
                            nc.tensor.matmul(
                                psW[:, cj, :],
                                h1T[:, cj * P : (cj + 1) * P],
                                W1s[:],
                                start=True,
                                stop=True,
                            )
                        hw = hp.tile([P, NCH, H], f32r, tag="hw")
                        for cj in range(NCH):
                            nc.scalar.activation(
                                hw[:, cj, :],
                                psW[:, cj, :],
                                AF.Copy,
                                scale=maskb[:, cj : cj + 1],
                            )
                    else:
                        L2s = medp.tile([H, N], dt.float32, tag="L2s")
                        nc.scalar.activation(
                            L2s[:], psL[:], AF.Identity, bias=b1col[:, 0:1]
                        )
                        psN = psB.tile([P, NCH, H], dt.float32, tag="mm")
                        for ci in range(NCH):
                            nc.tensor.transpose(
                                psN[:, ci, :],
                                L2s[:, ci * P : (ci + 1) * P],
                                identF[:],
                            )
                        h2 = hp.tile([P, NCH, H], dt.float32, tag="h2")
                        if USE_SILU:
                            nc.scalar.activation(h2[:], psN[:], AF.Silu)
                        else:
                            sg2 = hp.tile([P, NCH, H], dt.float32, tag="sg2")
                            nc.scalar.activation(sg2[:], psN[:], AF.Sigmoid)
                            nc.vector.tensor_tensor(
                                h2[:], psN[:], sg2[:], ALU.mult
                            )

                # masked mean pool; per-graph embedding -> gAll row
                psG = psD.tile([1, H], dt.float32, tag="head")
                for ci in range(NCH):
                    nc.tensor.matmul(
                        psG[:],
                        maskdiv[:, ci : ci + 1],
                        h2[:, ci, :],
                        start=(ci == 0),
                        stop=(ci == NCH - 1),
                    )
                nc.scalar.copy(gAll[:, b * H : (b + 1) * H], psG[:])

            # 3-stage software pipeline: dma(b) | deg-chain(b-1) | compute(b-2)
            # 5-stage pipeline: dma(b) | transfer in flight | folds(b-1)
            # on gpsimd | deg-math(b-2) on DVE/ACT | compute(b-3).  Each
            # stage's inputs were produced a full iteration earlier, so no
            # engine ever stalls mid-queue on another engine's current
            # work; emission order puts compute first in every queue.
            sts = {}
            for b in range(n_batches + 3):
                if b >= 3:
                    stage_compute(b - 3, sts.pop(b - 3))
                if 1 <= b < n_batches + 1:
                    stage_folds(b - 1, sts[b - 1])
                if b < n_batches:
                    sts[b] = dma_issue(b)
                if 2 <= b < n_batches + 2:
                    stage_degmath(b - 2, sts[b - 2])

            # batched MLP head over all graphs: [H, B] columns
            psGT = psD.tile([H, n_batches], dt.float32, tag="head2")
            for b in range(n_batches):
                nc.tensor.transpose(
                    psGT[:, b : b + 1],
                    gAll[:, b * H : (b + 1) * H],
                    identF[0:1, 0:1],
                )
            gT = constp.tile([H, n_batches], dt.float32)
            nc.scalar.copy(gT[:], psGT[:])
            psH1 = psD.tile([H, n_batches], dt.float32, tag="head2")
            nc.tensor.matmul(psH1[:], Wl1s[:], gT[:], start=True, stop=True)
            g1 = constp.tile([H, n_batches], dt.float32)
            if USE_SILU:
                nc.scalar.activation(g1[:], psH1[:], AF.Silu, bias=bl1c[:, 0:1])
            else:
                g1s = constp.tile([H, n_batches], dt.float32)
                nc.scalar.activation(
                    g1s[:], psH1[:], AF.Sigmoid, bias=bl1c[:, 0:1]
                )
                nc.vector.scalar_tensor_tensor(
                    g1[:], psH1[:], bl1c[:, 0:1], g1s[:],
                    op0=ALU.add, op1=ALU.mult,
                )
            psO = psD.tile([OUT, n_batches], dt.float32, tag="head2")
            nc.tensor.matmul(psO[:], Wl2s[:], g1[:], start=True, stop=True)
            nc.scalar.activation(outS[:], psO[:], AF.Identity, bias=bl2c[:, 0:1])

            nc.sync.dma_start(out_d.ap().rearrange("b c -> c b"), outS[:])

    nc.compile()
    return nc


_NC_CACHE = {}


def _get_nc(n_batches=BPC):
    if n_batches not in _NC_CACHE:
        _NC_CACHE[n_batches] = build_nc(n_batches)
    return _NC_CACHE[n_batches]


def make_in_maps(x, adj, mask, W0, b0, W1, b1, Wl1, bl1, Wl2, bl2):
    ws = dict(
        W0=np.ascontiguousarray(W0, np.float32),
        b0=np.ascontiguousarray(b0, np.float32),
        W1=np.ascontiguousarray(W1, np.float32),
        b1=np.ascontiguousarray(b1, np.float32),
        Wl1=np.ascontiguousarray(Wl1, np.float32),
        bl1=np.ascontiguousarray(bl1, np.float32),
        Wl2=np.ascontiguousarray(Wl2, np.float32),
        bl2=np.ascontiguousarray(bl2, np.float32),
    )
    in_maps = []
    for c in range(NCORES):
        sl = slice(c * BPC, (c + 1) * BPC)
        m = dict(
            adj=np.ascontiguousarray(adj[sl], np.float32),
            x=np.ascontiguousarray(x[sl], np.float32),
            mask=np.ascontiguousarray(mask[sl], np.int32),
        )
        m.update(ws)
        in_maps.append(m)
    return in_maps


def kernel(x, adj, mask, W0, b0, W1, b1, Wl1, bl1, Wl2, bl2, **kw):
    nc = _get_nc()
    in_maps = make_in_maps(x, adj, mask, W0, b0, W1, b1, Wl1, bl1, Wl2, bl2)
    res = run_bass_kernel_spmd(nc, in_maps, core_ids=list(range(NCORES)))
    out = np.concatenate([res.results[c]["out"] for c in range(NCORES)], axis=0)
    return out.astype(np.float32)


# revision 23
# speedup vs baseline: 1.1485x; 1.0636x over previous
"""DenseGCN (multi-edge-type) Trainium2 kernel.

Data-parallel over batch across 8 NeuronCores (8 graphs per core).

Math (per graph):
  adj_sl = adj with diagonal set to 1 (self loops), per edge type f
  deg[i,f] = clip(sum_j adj_sl[i,j,f], 1)^-0.5
  layer(h) = silu((sum_f D_f adj_sl_f D_f) @ (h W) + b) * mask
  Collapse edge types:  A2[i,j] = sum_f deg[i,f]*adj[i,j,f]*deg[j,f]
  self-loop correction as a diagonal add: A2full = A2 + diag(Cs),
  Cs[i] = sum_f deg[i,f]^2 * (1 - adj[i,i,f]).

Engine plan (per graph, 2-stage software pipeline across graphs):
  DMA:    one casting DMA loads adj fp32->bf16 [P, NCH, N, F] (gpsimd
          SWDGE initiates; HBM bytes unchanged, SBUF halved, no DVE
          rowscale/convert needed).
  GPSIMD: degree row-sums as contiguous pairwise fold tree, levels 1-3
          (512->64 j's), fp32 partials; the idle engine absorbs the
          1M-elem read that used to cost ~160us on ACT+DVE.
  DVE:    fold tail (64->1), deg = rsqrt via fast-inverse-sqrt + 2
          Newton steps (fp32, exact to ~4e-6), hi/lo split of deg,
          one batched TT builds all 32 diag matrices + 4 Cs diag
          blocks, assembly stt (f=1..3), small mask/Cs ops.
  PE:     transpose+deg_i-scale fused: per 128x128 block, 2 matmuls
          (stationary = raw bf16 adj block, moving = diag(hi(deg_i)),
          diag(lo(deg_i))) accumulating in PSUM -- bf16 speed, fp32
          accuracy (hi+lo splits deg into two bf16 values).
          Layers as before: psL[H,i] = sum_cj h^T A2T in f32r.
  ACT:    assembly f=0 scale-copy, Silu activations (single op instead
          of Sigmoid+DVE-mult), x/h0 copies, mask-scaled PSUM->SBUF.
"""

import os

import numpy as np

import concourse.bass as bass
from concourse import bacc, masks, mybir, tile
from concourse.bass_utils import run_bass_kernel_spmd

B, N, F = 64, 512, 4
IN, H, OUT = 64, 128, 16
NCORES = 8
BPC = B // NCORES  # graphs per core
P = 128
NCH = N // P  # 4 chunks of 128 nodes

dt = mybir.dt
AF = mybir.ActivationFunctionType
ALU = mybir.AluOpType
AXL = mybir.AxisListType

MAGIC = 0x5F3759DF

# knobs for A/B on hardware
GP_FOLDS = int(os.environ.get("GCN_GP_FOLDS", "3"))  # fold levels on gpsimd
NEWTON = int(os.environ.get("GCN_NEWTON", "2"))
# AF.Silu is not implemented in CoreSim; set GCN_SILU=0 to validate in sim
USE_SILU = os.environ.get("GCN_SILU", "1") == "1"


def build_nc(n_batches=BPC):
    nc = bacc.Bacc(
        "TRN2", target_bir_lowering=False, debug=False, enable_asserts=False
    )

    adj_d = nc.dram_tensor(
        "adj", [n_batches, N, N, F], dt.float32, kind="ExternalInput"
    )
    x_d = nc.dram_tensor("x", [n_batches, N, IN], dt.float32, kind="ExternalInput")
    mask_d = nc.dram_tensor("mask", [n_batches, N], dt.int32, kind="ExternalInput")
    W0_d = nc.dram_tensor("W0", [IN, H], dt.float32, kind="ExternalInput")
    b0_d = nc.dram_tensor("b0", [H], dt.float32, kind="ExternalInput")
    W1_d = nc.dram_tensor("W1", [H, H], dt.float32, kind="ExternalInput")
    b1_d = nc.dram_tensor("b1", [H], dt.float32, kind="ExternalInput")
    Wl1_d = nc.dram_tensor("Wl1", [H, H], dt.float32, kind="ExternalInput")
    bl1_d = nc.dram_tensor("bl1", [H], dt.float32, kind="ExternalInput")
    Wl2_d = nc.dram_tensor("Wl2", [H, OUT], dt.float32, kind="ExternalInput")
    bl2_d = nc.dram_tensor("bl2", [OUT], dt.float32, kind="ExternalInput")
    out_d = nc.dram_tensor("out", [n_batches, OUT], dt.float32, kind="ExternalOutput")

    f32r = dt.float32r
    bf16 = dt.float16  # fp16: 10 mantissa bits, same PE speed as bf16

    with tile.TileContext(nc) as tc:
        with (
            tc.tile_pool(name="const", bufs=1) as constp,
            tc.tile_pool(name="adjp", bufs=5) as adjp,
            tc.tile_pool(name="a2p", bufs=2 * NCH) as a2p,
            tc.tile_pool(name="hp", bufs=3) as hp,
            tc.tile_pool(name="smallp", bufs=4) as smallp,
            tc.tile_pool(name="medp", bufs=2) as medp,
            tc.tile_pool(name="dsp", bufs=3) as dsp,
            tc.tile_pool(name="dsl", bufs=2) as dsl,
            tc.tile_pool(name="diagp", bufs=2) as diagp,
            tc.tile_pool(name="psA", bufs=2, space="PSUM") as psA,
            tc.tile_pool(name="psB", bufs=3, space="PSUM") as psB,
            tc.tile_pool(name="psC", bufs=1, space="PSUM") as psC,
            tc.tile_pool(name="psD", bufs=1, space="PSUM") as psD,
        ):
            identF = constp.tile([P, P], dt.float32)
            masks.make_identity(nc, identF[:])
            identB = constp.tile([P, P], bf16)
            nc.vector.tensor_copy(identB[:], identF[:])

            W0s = constp.tile([IN, H], dt.float32)
            nc.sync.dma_start(W0s[:], W0_d.ap())
            W1s = constp.tile([H, H], dt.float32)
            nc.sync.dma_start(W1s[:], W1_d.ap())
            Wl1s = constp.tile([H, H], dt.float32)
            nc.sync.dma_start(Wl1s[:], Wl1_d.ap())
            Wl2s = constp.tile([H, OUT], dt.float32)
            nc.sync.dma_start(Wl2s[:], Wl2_d.ap())
            b0col = constp.tile([H, 1], dt.float32)
            nc.sync.dma_start(b0col[:], b0_d.ap().rearrange("(p o) -> p o", o=1))
            b1col = constp.tile([H, 1], dt.float32)
            nc.sync.dma_start(b1col[:], b1_d.ap().rearrange("(p o) -> p o", o=1))
            bl1c = constp.tile([H, 1], dt.float32)
            nc.sync.dma_start(bl1c[:], bl1_d.ap().rearrange("(p o) -> p o", o=1))
            bl2c = constp.tile([OUT, 1], dt.float32)
            nc.sync.dma_start(bl2c[:], bl2_d.ap().rearrange("(p o) -> p o", o=1))
            gAll = constp.tile([1, n_batches * H], dt.float32)
            outS = constp.tile([OUT, n_batches], dt.float32)

            def dma_issue(b):
                st = {}
                # whole-graph adj as bf16 (DMA converts): [i-part, ci, j, f]
                adjB = adjp.tile([P, NCH, N, F], bf16, tag="adjB")
                nc.gpsimd.dma_start(
                    adjB[:],
                    adj_d.ap()[b].rearrange("(c p) j f -> p c j f", p=P),
                )
                st["adjB"] = adjB
                diagN = smallp.tile([P, NCH, F], dt.float32, tag="diag")
                nc.sync.dma_start(
                    diagN[:],
                    bass.AP(
                        tensor=adj_d,
                        offset=b * N * N * F,
                        ap=[[(N * F + F), P], [(N * F + F) * P, NCH], [1, F]],
                    ),
                )
                xb = smallp.tile([P, NCH, IN], dt.float32, tag="xb")
                nc.sync.dma_start(
                    xb[:], x_d.ap()[b].rearrange("(c p) d -> p c d", p=P)
                )
                mi = smallp.tile([P, NCH], dt.int32, tag="mi")
                nc.sync.dma_start(
                    mi[:], mask_d.ap()[b].rearrange("(c p) -> p c", p=P)
                )
                st["diagN"] = diagN
                st["xb"] = xb
                st["mi"] = mi
                return st

            def stage_folds(b, st):
                # gpsimd-only: fold levels 1-3 (512 -> 64 j's).  Runs one
                # pipeline stage ahead of stage_degmath so the DVE tail
                # never waits on gpsimd mid-queue.  Early-level partials in
                # fp16 (error contribution ~1e-4 on a ~256 sum).
                adjB = st["adjB"]
                hdt = dt.float16
                ds1 = dsp.tile([P, NCH, (N // 4) * F], hdt, tag="ds1")
                half = N // 2
                for ci in range(NCH):
                    t1 = dsl.tile([P, half * F], hdt, tag="t1")
                    nc.gpsimd.tensor_tensor(
                        t1[:],
                        adjB[:, ci, 0:half, :].rearrange("p j f -> p (j f)"),
                        adjB[:, ci, half:N, :].rearrange("p j f -> p (j f)"),
                        ALU.add,
                    )
                    nc.gpsimd.tensor_tensor(
                        ds1[:, ci, :],
                        t1[:, 0 : (half // 2) * F],
                        t1[:, (half // 2) * F : half * F],
                        ALU.add,
                    )
                st["ds1"] = ds1

            def stage_degmath(b, st):
                adjB = st["adjB"]
                diagN = st["diagN"]
                xb = st["xb"]
                mi = st["mi"]
                maskb = smallp.tile([P, NCH], dt.float32, tag="maskb")
                nc.vector.tensor_copy(maskb[:], mi[:])
                st["maskb"] = maskb
                maskdiv = smallp.tile([P, NCH], dt.float32, tag="md")
                nc.vector.tensor_scalar_mul(maskdiv[:], maskb[:], 1.0 / N)
                st["maskdiv"] = maskdiv

                # DVE fold tail 128 -> 1 j's (fp16 2x for the wide levels,
                # fp32 from width 32 down)
                cur = st.pop("ds1")
                w = N // 4
                while w > 1:
                    odt = dt.float16 if w > 16 else dt.float32
                    nxt = dsl.tile([P, NCH, (w // 2) * F], odt, tag=f"ds{w}")
                    nc.vector.tensor_tensor(
                        nxt[:],
                        cur[:, :, 0 : (w // 2) * F],
                        cur[:, :, (w // 2) * F : w * F],
                        ALU.add,
                    )
                    cur, w = nxt, w // 2
                degsum = cur  # [P, NCH, F]

                # dtmp = max(degsum + 1 - diag, 1)
                dtmp = smallp.tile([P, NCH, F], dt.float32, tag="dtmp")
                nc.vector.tensor_tensor(
                    dtmp[:], degsum[:], diagN[:], ALU.subtract
                )
                nc.vector.tensor_scalar(dtmp[:], dtmp[:], 1.0, 1.0, ALU.add, ALU.max)
                # deg = dtmp^-0.5 fast-inverse-sqrt + Newton (DVE, fp32)
                ti = smallp.tile([P, NCH, F], dt.int32, tag="ti")
                nc.vector.tensor_scalar(
                    ti[:], dtmp[:].bitcast(dt.int32), 1, None, ALU.arith_shift_right
                )
                nc.vector.tensor_scalar(ti[:], ti[:], -1, MAGIC, ALU.mult, ALU.add)
                e = smallp.tile([P, NCH, F], dt.float32, tag="nwt")
                deg = ti[:].bitcast(dt.float32)
                for _ in range(NEWTON):
                    nc.vector.tensor_tensor(e[:], dtmp[:], deg, ALU.mult)
                    nc.vector.tensor_tensor(e[:], e[:], deg, ALU.mult)
                    nc.vector.tensor_scalar(
                        e[:], e[:], -0.5, 1.5, ALU.mult, ALU.add
                    )
                    nc.vector.tensor_tensor(deg, deg, e[:], ALU.mult)
                st["deg"] = deg

                # Cs = sum_f deg^2 * (1 - diag)
                om = smallp.tile([P, NCH, F], dt.float32, tag="om")
                nc.scalar.activation(
                    om[:], diagN[:], AF.Copy, scale=-1.0, bias=1.0
                )
                csf = smallp.tile([P, NCH, F], dt.float32, tag="csf")
                nc.vector.tensor_tensor(csf[:], deg, deg, ALU.mult)
                nc.vector.tensor_tensor(csf[:], csf[:], om[:], ALU.mult)
                Cs = smallp.tile([P, NCH], dt.float32, tag="Cs")
                nc.vector.tensor_reduce(Cs[:], csf[:], axis=AXL.X, op=ALU.add)

                # hi/lo split of deg into bf16 pair (exact to ~2^-16)
                NG = NCH * F
                degB = smallp.tile([P, NCH, F], bf16, tag="degB")
                nc.scalar.copy(degB[:], deg)
                hiF = smallp.tile([P, NCH, F], dt.float32, tag="hiF")
                nc.scalar.copy(hiF[:], degB[:])
                loF = smallp.tile([P, NCH, F], dt.float32, tag="loF")
                nc.vector.tensor_tensor(loF[:], deg, hiF[:], ALU.subtract)

                # diagonal build: diagHL[p, hl, g, q] = ident[p, q] *
                # (hl ? lo : hi)[p, g], g = (ci, f).  hi half: 16 ACT
                # scale-copies (ACT has slack); lo half: one batched DVE TT.
                diagHL = diagp.tile([P, 2, NG, P], bf16, tag="diagHL")
                for ci in range(NCH):
                    for f in range(F):
                        nc.scalar.activation(
                            diagHL[:, 0, ci * F + f, :],
                            identB[:],
                            AF.Copy,
                            scale=hiF[:, ci, f : f + 1],
                        )
                loFf = loF[:].rearrange("p c f -> p (c f)")
                nc.vector.tensor_tensor(
                    diagHL[:, 1, :, :],
                    identB[:, None, :].to_broadcast([P, NG, P]),
                    loFf[:, :, None].to_broadcast([P, NG, P]),
                    ALU.mult,
                )
                st["diagHL"] = diagHL
                # Cs diagonal blocks (bf16)
                csdB = diagp.tile([P, NCH, P], bf16, tag="csdB")
                nc.vector.tensor_tensor(
                    csdB[:],
                    identB[:, None, :].to_broadcast([P, NCH, P]),
                    Cs[:, :, None].to_broadcast([P, NCH, P]),
                    ALU.mult,
                )
                st["csdB"] = csdB

                # h0 = x @ W0 (natural [j, H] layout, exact fp32)
                psX = psC.tile([IN, N], dt.float32, tag="px")
                for ci in range(NCH):
                    nc.tensor.transpose(
                        psX[:, ci * P : (ci + 1) * P], xb[:, ci, :], identF[:]
                    )
                xTs = medp.tile([IN, N], dt.float32, tag="xTs")
                nc.scalar.copy(xTs[:], psX[:])
                psH0 = psC.tile([P, NCH, H], dt.float32, tag="px")
                for ci in range(NCH):
                    nc.tensor.matmul(
                        psH0[:, ci, :],
                        xTs[:, ci * P : (ci + 1) * P],
                        W0s[:],
                        start=True,
                        stop=True,
                    )
                h0 = hp.tile([P, NCH, H], f32r, tag="h0")
                nc.scalar.copy(h0[:], psH0[:])
                st["h0"] = h0

            def stage_compute(b, st):
                adjB = st["adjB"]
                deg = st["deg"]
                diagHL = st["diagHL"]
                csdB = st["csdB"]
                maskb = st["maskb"]
                maskdiv = st["maskdiv"]

                # transpose + deg_i scale (hi/lo diag matmuls) + assemble
                # A2T [j part, i free] (+ Cs diag)
                # Per cj, two partial accumulators: accA = deg0*BT0 +
                # deg1*BT1 (+ Cs diag), accB = deg2*BT2 + deg3*BT3.  The
                # f=0/f=2 scale-copies run on ACT, f=1/f=3 stt on DVE; the
                # layer matmuls accumulate both halves in PSUM, so no final
                # merge op is needed.
                A2Ta, A2Tb = [], []
                for cj in range(NCH):
                    accA = a2p.tile([P, N], f32r, tag="A2Ta")
                    accB = a2p.tile([P, N], f32r, tag="A2Tb")
                    for f in range(F):
                        acc = accA if f < 2 else accB
                        BT = psA.tile([P, N], dt.float32, tag="BT")
                        for ci in range(NCH):
                            blk = adjB[:, ci, cj * P : (cj + 1) * P, f]
                            out = BT[:, ci * P : (ci + 1) * P]
                            g = ci * F + f
                            nc.tensor.matmul(
                                out, blk, diagHL[:, 0, g, :],
                                start=True, stop=False,
                            )
                            nc.tensor.matmul(
                                out, blk, diagHL[:, 1, g, :],
                                start=False, stop=True,
                            )
                        if f % 2 == 0:
                            nc.scalar.activation(
                                acc[:], BT[:], AF.Copy, scale=deg[:, cj, f : f + 1]
                            )
                        else:
                            nc.vector.scalar_tensor_tensor(
                                acc[:], BT[:], deg[:, cj, f : f + 1], acc[:],
                                op0=ALU.mult, op1=ALU.add,
                            )
                    nc.vector.tensor_tensor(
                        accA[:, cj * P : (cj + 1) * P],
                        accA[:, cj * P : (cj + 1) * P],
                        csdB[:, cj, :],
                        ALU.add,
                    )
                    A2Ta.append(accA)
                    A2Tb.append(accB)

                # two GCN layers, transposed [H, i] layout
                hw = st["h0"]
                for l in range(2):
                    psL = psB.tile([H, N], dt.float32, tag="mm")
                    for cj in range(NCH):
                        for A2T in (A2Ta, A2Tb):
                            nc.tensor.matmul(
                                psL[:],
                                hw[:, cj, :],
                                A2T[cj][:],
                                start=(cj == 0 and A2T is A2Ta),
                                stop=(cj == NCH - 1 and A2T is A2Tb),
                            )
                    if l == 0:
                        h1T = medp.tile([H, N], dt.float32, tag="h1T")
                        if USE_SILU:
                            nc.scalar.activation(
                                h1T[:], psL[:], AF.Silu, bias=b0col[:, 0:1]
                            )
                        else:
                            sg1 = medp.tile([H, N], dt.float32, tag="sg1")
                            nc.scalar.activation(
                                sg1[:], psL[:], AF.Sigmoid, bias=b0col[:, 0:1]
                            )
                            nc.vector.scalar_tensor_tensor(
                                h1T[:], psL[:], b0col[:, 0:1], sg1[:],
                                op0=ALU.add, op1=ALU.mult,
                            )
                        psW = psB.tile([P, NCH, H], dt.float32, tag="mm")
                        for cj in range(NCH):
                            nc.tensor.matmul(
                                psW[:, cj, :],
                                h1T[:, cj * P : (cj + 1) * P],
                                W1s[:],
                                start=True,
                                stop=True,
                            )
                        hw = hp.tile([P, NCH, H], f32r, tag="hw")
                        for cj in range(NCH):
                            nc.scalar.activation(
                                hw[:, cj, :],
                                psW[:, cj, :],
                                AF.Copy,
                                scale=maskb[:, cj : cj + 1],
                            )
                    else:
                        L2s = medp.tile([H, N], dt.float32, tag="L2s")
                        nc.scalar.activation(
                            L2s[:], psL[:], AF.Identity, bias=b1col[:, 0:1]
                        )
                        psN = psB.tile([P, NCH, H], dt.float32, tag="mm")
                        for ci in range(NCH):
                            nc.tensor.transpose(
                                psN[:, ci, :],
                                L2s[:, ci * P : (ci + 1) * P],
                                identF[:],
                            )
                        h2 = hp.tile([P, NCH, H], dt.float32, tag="h2")
                        if USE_SILU:
                            nc.scalar.activation(h2[:], psN[:], AF.Silu)
                        else:
                            sg2 = hp.tile([P, NCH, H], dt.float32, tag="sg2")
                            nc.scalar.activation(sg2[:], psN[:], AF.Sigmoid)
                            nc.vector.tensor_tensor(
                                h2[:], psN[:], sg2[:], ALU.mult
                            )

                # masked mean pool; per-graph embedding -> gAll row
                psG = psD.tile([1, H], dt.float32, tag="head")
                for ci in range(NCH):
                    nc.tensor.matmul(
                        psG[:],
                        maskdiv[:, ci : ci + 1],
                        h2[:, ci, :],
                        start=(ci == 0),
                        stop=(ci == NCH - 1),
                    )
                nc.scalar.copy(gAll[:, b * H : (b + 1) * H], psG[:])

            # 3-stage software pipeline: dma(b) | deg-chain(b-1) | compute(b-2)
            # 5-stage pipeline: dma(b) | transfer in flight | folds(b-1)
            # on gpsimd | deg-math(b-2) on DVE/ACT | compute(b-3).  Each
            # stage's inputs were produced a full iteration earlier, so no
            # engine ever stalls mid-queue on another engine's current
            # work; emission order puts compute first in every queue.
            sts = {}
            for b in range(n_batches + 3):
                if b >= 3:
                    stage_compute(b - 3, sts.pop(b - 3))
                if 1 <= b < n_batches + 1:
                    stage_folds(b - 1, sts[b - 1])
                if b < n_batches:
                    sts[b] = dma_issue(b)
                if 2 <= b < n_batches + 2:
                    stage_degmath(b - 2, sts[b - 2])

            # batched MLP head over all graphs: [H, B] columns
            psGT = psD.tile([H, n_batches], dt.float32, tag="head2")
            for b in range(n_batches):
                nc.tensor.transpose(
                    psGT[:, b : b + 1],
                    gAll[:, b * H : (b + 1) * H],
                    identF[0:1, 0:1],
                )
            gT = constp.tile([H, n_batches], dt.float32)
            nc.scalar.copy(gT[:], psGT[:])
            psH1 = psD.tile([H, n_batches], dt.float32, tag="head2")
            nc.tensor.matmul(psH1[:], Wl1s[:], gT[:], start=True, stop=True)
            g1 = constp.tile([H, n_batches], dt.float32)
            if USE_SILU:
                nc.scalar.activation(g1[:], psH1[:], AF.Silu, bias=bl1c[:, 0:1])
            else:
                g1s = constp.tile([H, n_batches], dt.float32)
                nc.scalar.activation(
                    g1s[:], psH1[:], AF.Sigmoid, bias=bl1c[:, 0:1]
                )
                nc.vector.scalar_tensor_tensor(
                    g1[:], psH1[:], bl1c[:, 0:1], g1s[:],
                    op0=ALU.add, op1=ALU.mult,
                )
            psO = psD.tile([OUT, n_batches], dt.float32, tag="head2")
            nc.tensor.matmul(psO[:], Wl2s[:], g1[:], start=True, stop=True)
            nc.scalar.activation(outS[:], psO[:], AF.Identity, bias=bl2c[:, 0:1])

            nc.sync.dma_start(out_d.ap().rearrange("b c -> c b"), outS[:])

    nc.compile()
    return nc


_NC_CACHE = {}


def _get_nc(n_batches=BPC):
    if n_batches not in _NC_CACHE:
        _NC_CACHE[n_batches] = build_nc(n_batches)
    return _NC_CACHE[n_batches]


def make_in_maps(x, adj, mask, W0, b0, W1, b1, Wl1, bl1, Wl2, bl2):
    ws = dict(
        W0=np.ascontiguousarray(W0, np.float32),
        b0=np.ascontiguousarray(b0, np.float32),
        W1=np.ascontiguousarray(W1, np.float32),
        b1=np.ascontiguousarray(b1, np.float32),
        Wl1=np.ascontiguousarray(Wl1, np.float32),
        bl1=np.ascontiguousarray(bl1, np.float32),
        Wl2=np.ascontiguousarray(Wl2, np.float32),
        bl2=np.ascontiguousarray(bl2, np.float32),
    )
    in_maps = []
    for c in range(NCORES):
        sl = slice(c * BPC, (c + 1) * BPC)
        m = dict(
            adj=np.ascontiguousarray(adj[sl], np.float32),
            x=np.ascontiguousarray(x[sl], np.float32),
            mask=np.ascontiguousarray(mask[sl], np.int32),
        )
        m.update(ws)
        in_maps.append(m)
    return in_maps


def kernel(x, adj, mask, W0, b0, W1, b1, Wl1, bl1, Wl2, bl2, **kw):
    nc = _get_nc()
    in_maps = make_in_maps(x, adj, mask, W0, b0, W1, b1, Wl1, bl1, Wl2, bl2)
    res = run_bass_kernel_spmd(nc, in_maps, core_ids=list(range(NCORES)))
    out = np.concatenate([res.results[c]["out"] for c in range(NCORES)], axis=0)
    return out.astype(np.float32)
